# revision 19
# baseline (speedup 1.0000x reference)
"""Trainium2 Bass kernel for the hybrid attention head (nn_AttentionHead_Hybrid).

Math (per batch):
    norms  n_i = ||x_i||;  xh = x / n
    O      = product of 2016 Givens rotations (built on host, fp32)
    S[i,j] = xh_i . O . xh_j
    A      = S^2 * n_i n_j ;  P = softmax(A / 8)
    V      = x @ Vw^T + Vb
    out    = LayerNorm(P @ V + x) * gamma + beta

Device formulation (per core, 4 batches):
    W'     = diag(s') X with s'_n = ||x_n||^-1/2 * 8^-1/4 * (128/ln2)^1/4
    gt     = (W' O)^T  (host-prepped, f16)          so R[j,i] = w'_j O^T w'_i
    R^2    = (128/ln2) * A/8
    E      = exp(A/8) via Schraudolph: bitcast_bf16(int16(R^2 + B))  [one DVE
             tensor_scalar in 4x mode; the sawtooth error cancels in softmax]
    Vt     = [X Vw^T | 1]  (bf16; ones column gives softmax row-sums for free;
             V_b folded into the residual on host: xr = x + V_b)
    OUT^T  = sum_j Vt[j,:]^T E^T[j,:]   in [65, N] psum -> DMA to SBUF ->
             PE-transpose back -> y = OUT*rcol + xr -> LayerNorm (bn_stats)
    out    bf16, converted to f32 on host.

Sharding: data-parallel over batch, 4 batches per core on 8 cores.
"""

import math

import numpy as np
import ml_dtypes

import concourse.bacc as bacc
import concourse.bass as bass
import concourse.tile as tile
from concourse import bass_utils, mybir

AF = mybir.ActivationFunctionType
ALU = mybir.AluOpType
DT = mybir.dt

B, N, D = 32, 1024, 64
NCORES = 8
BPC = B // NCORES          # batches per core
NT = N // 128              # 128-row tiles per batch

SCHRAUD_C4 = 128.0 / math.log(2.0)          # scale absorbed into W' (4th root)
SCHRAUD_B = 16256.0 - 128.0 * 0.0430        # bf16 exponent bias, sigma-centered

# of the 8 per-batch [128,1024] squares, how many run on ACT (rest on DVE)
K_ACT_SQ = 8


def _build_orthogonal(phi: np.ndarray, d: int = D) -> np.ndarray:
    """fp32 replica of the reference jax.lax.scan Givens chain."""
    O = np.eye(d, dtype=np.float32)
    ii, jj = np.triu_indices(d, k=1)
    c = np.cos(phi.astype(np.float32))
    s = np.sin(phi.astype(np.float32))
    for k in range(len(phi)):
        i, j = int(ii[k]), int(jj[k])
        ri = O[i].copy()
        rj = O[j].copy()
        O[i] = c[k] * ri + s[k] * rj
        O[j] = -s[k] * ri + c[k] * rj
    return O


def _build_nc(apply_gamma_beta: bool):
    nc = bacc.Bacc("TRN2", target_bir_lowering=False)

    wt_t = nc.dram_tensor("wt", [BPC, D, N], DT.float16, kind="ExternalInput")
    gt_t = nc.dram_tensor("gt", [BPC, D, N], DT.float16, kind="ExternalInput")
    xo_t = nc.dram_tensor("xo", [BPC, D, N], DT.float16, kind="ExternalInput")
    xr_t = nc.dram_tensor("xr", [BPC, N, D], DT.bfloat16, kind="ExternalInput")
    xs_t = nc.dram_tensor("xs", [BPC, N], DT.float32, kind="ExternalInput")
    vw_t = nc.dram_tensor("vw", [D, D + 1], DT.float16, kind="ExternalInput")
    id_t = nc.dram_tensor("ident", [128, 128], DT.float32, kind="ExternalInput")
    gb_t = nc.dram_tensor("gb", [2, D], DT.float32, kind="ExternalInput")
    out_t = nc.dram_tensor("out", [BPC, N, D], DT.bfloat16, kind="ExternalOutput")

    with tile.TileContext(nc) as tc:
        with (
            tc.tile_pool(name="const", bufs=1) as constp,
            tc.tile_pool(name="loadp", bufs=2) as loadp,
            tc.tile_pool(name="asqp", bufs=3) as asqp,
            tc.tile_pool(name="ep", bufs=3) as ep,
            tc.tile_pool(name="vp", bufs=2) as vp,
            tc.tile_pool(name="otp", bufs=2) as otp,
            tc.tile_pool(name="yp", bufs=2) as yp,
            tc.tile_pool(name="outp", bufs=2) as outp,
            tc.tile_pool(name="statp", bufs=2) as statp,
            tc.tile_pool(name="psA", bufs=3, space="PSUM") as psA,
            tc.tile_pool(name="psB", bufs=1, space="PSUM") as psB,
        ):
            vwb_sb = constp.tile([128, D + 1], DT.float16)
            nc.sync.dma_start(out=vwb_sb[0:D, :], in_=vw_t[:, :])
            nc.sync.dma_start(out=vwb_sb[D : 2 * D, :], in_=vw_t[:, :])
            id_sb = constp.tile([128, 128], DT.float32)
            nc.sync.dma_start(out=id_sb, in_=id_t[:, :])
            if apply_gamma_beta:
                gam_sb = constp.tile([128, D], DT.float32)
                nc.sync.dma_start(out=gam_sb, in_=gb_t[0, :].to_broadcast([128, D]))
                bet_sb = constp.tile([128, D], DT.float32)
                nc.sync.dma_start(out=bet_sb, in_=gb_t[1, :].to_broadcast([128, D]))

            # PE warm-up: dependency-free matmuls trip the HAM activity
            # window so real matmuls run at 2.4 GHz, not 1.2.
            warm = constp.tile([64, 512], DT.float16)
            nc.vector.memset(warm, 0.0)
            pw = psA.tile([128, 512], DT.float32, tag="pr", name="pw")
            for _ in range(8):
                nc.tensor.matmul(pw[0:64, :], lhsT=warm[:, 0:64], rhs=warm)

            def emit_loads(b):
                wt = loadp.tile([128, N], DT.float16, tag="wt")
                nc.sync.dma_start(out=wt[0:D, :], in_=wt_t[b])
                nc.sync.dma_start(out=wt[D : 2 * D, :], in_=wt_t[b])
                gt = loadp.tile([128, N], DT.float16, tag="gt")
                nc.sync.dma_start(out=gt[0:D, :], in_=gt_t[b])
                nc.sync.dma_start(out=gt[D : 2 * D, :], in_=gt_t[b])
                xo = loadp.tile([128, N], DT.float16, tag="xo")
                nc.sync.dma_start(out=xo[0:D, :], in_=xo_t[b])
                nc.sync.dma_start(out=xo[D : 2 * D, :], in_=xo_t[b])
                xr = loadp.tile([128, NT, D], DT.bfloat16, tag="xr")
                nc.sync.dma_start(
                    out=xr, in_=xr_t[b].rearrange("(t p) d -> p t d", p=128)
                )
                xs = loadp.tile([128, NT], DT.float32, tag="xs")
                nc.sync.dma_start(
                    out=xs, in_=xs_t[b].rearrange("(t p) -> p t", p=128)
                )
                return wt, gt, xo, xr, xs

            def emit_v(xo):
                # Vt = [X Vw^T | X vw_rowsum] (bias folded into residual on
                # host; the extra column yields sum_d num for the LN mean);
                # 64-contract pairs run concurrently in the two PE row groups.
                # Per-tile stride padded to 128 floats so no matmul output
                # crosses a PSUM bank boundary.
                # NB: tile_position=(64,0) matmuls fault at runtime unless the
                # PSUM output is bank-aligned, so these all run on group (0,0).
                pv = psA.tile([128, NT, 128], DT.float32, tag="pr", name="pv")
                for t in range(NT):
                    nc.tensor.matmul(
                        pv[:, t, 0 : D + 1],
                        lhsT=xo[0:D, t * 128 : (t + 1) * 128],
                        rhs=vwb_sb[0:D, :],
                        tile_position=(0, 0),
                    )
                v_sb = vp.tile([128, NT, 66], DT.bfloat16, tag="v")
                nc.vector.tensor_copy(v_sb[:, :, 0 : D + 1], pv[:, :, 0 : D + 1])
                nc.vector.memset(v_sb[:, :, D + 1], 1.0)
                return v_sb

            state = {0: emit_loads(0)}
            vg = {0: emit_v(state[0][2])}

            for b in range(BPC):
                wt, gt, xo, xr, xs = state[b]
                v_sb = vg[b]

                pot = psB.tile([128, N], DT.float32, tag="pot", name="pot")

                # ---- per pair of j-tiles: R (both PE row groups), then per
                # tile: square, schraudolph-exp, PV accumulate ----
                for q in range(4):
                    je, jo = 2 * q, 2 * q + 1
                    pre = psA.tile([128, N], DT.float32, tag="pr", name="pre")
                    pro = psA.tile([128, N], DT.float32, tag="pr", name="pro")
                    for c in range(2):
                        cs = slice(c * 512, (c + 1) * 512)
                        nc.tensor.matmul(
                            pre[:, cs],
                            lhsT=wt[0:D, je * 128 : (je + 1) * 128],
                            rhs=gt[0:D, cs],
                            tile_position=(0, 0),
                        )
                    for c in range(2):
                        cs = slice(c * 512, (c + 1) * 512)
                        nc.tensor.matmul(
                            pro[:, cs],
                            lhsT=wt[D : 2 * D, jo * 128 : (jo + 1) * 128],
                            rhs=gt[D : 2 * D, cs],
                            tile_position=(64, 0),
                        )
                    for jt, pr in ((je, pre), (jo, pro)):
                        asq = asqp.tile([128, N], DT.float16, tag="asq")
                        if jt < K_ACT_SQ:
                            nc.scalar.activation(asq, pr, AF.Square)
                        else:
                            nc.vector.tensor_tensor(
                                out=asq, in0=pr, in1=pr, op=ALU.mult
                            )
                        e = ep.tile([128, N], DT.int16, tag="e")
                        nc.vector.tensor_scalar(
                            e, asq, scalar1=SCHRAUD_B, scalar2=None, op0=ALU.add
                        )
                        eb = e.bitcast(DT.bfloat16)
                        for c in range(2):
                            nc.tensor.matmul(
                                pot[0 : D + 2, c * 512 : (c + 1) * 512],
                                lhsT=v_sb[:, jt, 0 : D + 2],
                                rhs=eb[:, c * 512 : (c + 1) * 512],
                                start=(jt == 0),
                                stop=(jt == NT - 1),
                            )
                    if q == 1 and b + 1 < BPC:
                        state[b + 1] = emit_loads(b + 1)
                    if q == 2 and b + 1 < BPC:
                        vg[b + 1] = emit_v(state[b + 1][2])

                # ---- OUT^T -> SBUF (half DVE, half ACT) ----
                ot = otp.tile([128, N], DT.float32, tag="ot")
                nc.vector.tensor_copy(ot[0 : D + 2, 0:512], pot[0 : D + 2, 0:512])
                nc.scalar.copy(ot[0 : D + 2, 512:N], pot[0 : D + 2, 512:N])

                # ---- transpose back; y = OUT*(1/rowsum) + xr ----
                # ptr cols: 0..63 num, 64 sum_d num, 65 denominator
                rcol = statp.tile([128, NT], DT.float32, tag="rcol")
                mus = statp.tile([128, NT], DT.float32, tag="mus")
                y = yp.tile([128, NT, D], DT.bfloat16, tag="y")
                for grp in range(2):
                    ptr = psA.tile([128, 4, D + 2], DT.float32, tag="pr", name="ptr")
                    for qq in range(4):
                        it = grp * 4 + qq
                        nc.tensor.transpose(
                            ptr[:, qq, :],
                            ot[0 : D + 2, it * 128 : (it + 1) * 128],
                            id_sb[0 : D + 2, 0 : D + 2],
                        )
                    g_sl = slice(grp * 4, grp * 4 + 4)
                    nc.vector.reciprocal_approx_fast(rcol[:, g_sl], ptr[:, :, D + 1])
                    rc4 = rcol[:, g_sl]
                    rcol_bc = bass.AP(
                        tensor=rcol.tensor, offset=rc4.offset,
                        ap=[rcol.ap[0], [1, 4], [0, D]],
                    )
                    nc.vector.tensor_tensor(
                        out=y[:, g_sl, :], in0=ptr[:, :, 0:D], in1=rcol_bc,
                        op=ALU.mult,
                    )
                    nc.vector.tensor_tensor(
                        out=mus[:, g_sl], in0=ptr[:, :, D], in1=rc4, op=ALU.mult
                    )
                nc.vector.tensor_add(y, y, xr)

                # ---- LayerNorm stats: mean from the free sum_d num column,
                # variance from one squared reduce ----
                ysq = yp.tile([128, NT, D], DT.bfloat16, tag="ysq")
                nc.vector.tensor_mul(ysq, y, y)
                ys2 = statp.tile([128, NT], DT.float32, tag="ys2")
                nc.vector.reduce_sum(ys2, ysq, axis=mybir.AxisListType.X)
                mu = statp.tile([128, NT], DT.float32, tag="mu")
                nc.gpsimd.tensor_scalar(
                    mu, mus, scalar1=1.0 / D, scalar2=None, op0=ALU.mult
                )
                nc.gpsimd.tensor_add(mu, mu, xs)
                # var + eps = ys2/64 - mu^2 + 1e-5
                m2 = statp.tile([128, NT], DT.float32, tag="m2")
                nc.gpsimd.tensor_mul(m2, mu, mu)
                ve = statp.tile([128, NT], DT.float32, tag="ve")
                nc.gpsimd.tensor_scalar(
                    ve, ys2, scalar1=1.0 / D, scalar2=1e-5, op0=ALU.mult, op1=ALU.add
                )
                nc.gpsimd.tensor_sub(ve, ve, m2)
                sd = statp.tile([128, NT], DT.float32, tag="sd")
                nc.scalar.activation(sd, ve, AF.Sqrt)
                rstd = statp.tile([128, NT], DT.float32, tag="rstd")
                nc.vector.reciprocal_approx_fast(rstd, sd)

                # ---- normalize per i-tile on Pool: out = (y - mu) * rstd ----
                out_sb = outp.tile([128, NT, D], DT.bfloat16, tag="o")
                for t in range(NT):
                    nc.gpsimd.tensor_scalar(
                        out_sb[:, t, :], y[:, t, :],
                        scalar1=mu[:, t : t + 1], scalar2=rstd[:, t : t + 1],
                        op0=ALU.subtract, op1=ALU.mult,
                    )
                if apply_gamma_beta:
                    for t in range(NT):
                        nc.gpsimd.tensor_mul(out_sb[:, t, :], out_sb[:, t, :], gam_sb)
                        nc.gpsimd.tensor_add(out_sb[:, t, :], out_sb[:, t, :], bet_sb)

                nc.sync.dma_start(
                    out=out_t[b].rearrange("(t p) d -> p t d", p=128), in_=out_sb
                )

    nc.compile()
    return nc


_NC_CACHE: dict = {}


def kernel(input1, V_w, V_b, phi, ln_gamma, ln_beta, _trace=False):
    input1 = np.ascontiguousarray(np.asarray(input1, dtype=np.float32))
    V_w = np.asarray(V_w, dtype=np.float32)
    V_b = np.asarray(V_b, dtype=np.float32)
    phi = np.asarray(phi, dtype=np.float32)
    ln_gamma = np.asarray(ln_gamma, dtype=np.float32)
    ln_beta = np.asarray(ln_beta, dtype=np.float32)

    apply_gb = not (np.all(ln_gamma == 1.0) and np.all(ln_beta == 0.0))

    if apply_gb not in _NC_CACHE:
        _NC_CACHE[apply_gb] = _build_nc(apply_gb)
    nc = _NC_CACHE[apply_gb]

    O = _build_orthogonal(phi)
    vwT = V_w.T.astype(np.float32)
    vw16 = np.ascontiguousarray(
        np.concatenate([vwT, vwT.sum(axis=1, keepdims=True)], axis=1)
        .astype(np.float16)
    )
    ident = np.eye(128, dtype=np.float32)
    gb = np.ascontiguousarray(np.stack([ln_gamma, ln_beta]).astype(np.float32))

    # host prep: W' = x * ||x||^-1/2 * 8^-1/4 * (128/ln2)^1/4 (transposed f16),
    # gt = (W' O)^T, xo = X^T, xr = x + V_b (bf16 residual w/ folded bias)
    ss = (input1.astype(np.float64) ** 2).sum(-1)
    s = (ss ** -0.25 * 8.0 ** -0.25 * SCHRAUD_C4 ** 0.25).astype(np.float32)
    w = input1 * s[..., None]
    wt_full = np.ascontiguousarray(w.transpose(0, 2, 1).astype(np.float16))
    gt_full = np.ascontiguousarray(
        (w @ O).transpose(0, 2, 1).astype(np.float16)
    )
    xo_full = np.ascontiguousarray(
        input1.transpose(0, 2, 1).astype(np.float16)
    )
    xr_f32 = input1 + V_b
    xr_full = np.ascontiguousarray(xr_f32.astype(ml_dtypes.bfloat16))
    xs_full = np.ascontiguousarray((xr_f32.sum(-1) / D).astype(np.float32))

    in_maps = []
    for c in range(NCORES):
        sl = slice(c * BPC, (c + 1) * BPC)
        in_maps.append(
            {
                "wt": wt_full[sl],
                "gt": gt_full[sl],
                "xo": xo_full[sl],
                "xr": xr_full[sl],
                "xs": xs_full[sl],
                "vw": vw16,
                "ident": ident,
                "gb": gb,
            }
        )

    res = bass_utils.run_bass_kernel_spmd(
        nc, in_maps, core_ids=list(range(NCORES)), trace=_trace
    )
    out = np.concatenate(
        [res.results[c]["out"].astype(np.float32) for c in range(NCORES)], axis=0
    )
    if _trace:
        kernel._last_result = res
    return out


# revision 20
# speedup vs baseline: 1.1101x; 1.1101x over previous
"""Trainium2 Bass kernel for the hybrid attention head (nn_AttentionHead_Hybrid).

Math (per batch):
    norms  n_i = ||x_i||;  xh = x / n
    O      = product of 2016 Givens rotations (built on host, fp32)
    S[i,j] = xh_i . O . xh_j
    A      = S^2 * n_i n_j ;  P = softmax(A / 8)
    V      = x @ Vw^T + Vb
    out    = LayerNorm(P @ V + x) * gamma + beta

Device formulation (per core, 4 batches):
    W'     = diag(s') X with s'_n = ||x_n||^-1/2 * 8^-1/4 * (128/ln2)^1/4
    gt     = (W' O)^T  (host-prepped, f16)          so R[j,i] = w'_j O^T w'_i
    R^2    = (128/ln2) * A/8
    E      = exp(A/8) via Schraudolph: bitcast_bf16(int16(R^2 + B))  [one DVE
             tensor_scalar in 4x mode; the sawtooth error cancels in softmax]
    Vt     = [X Vw^T | 1]  (bf16; ones column gives softmax row-sums for free;
             V_b folded into the residual on host: xr = x + V_b)
    OUT^T  = sum_j Vt[j,:]^T E^T[j,:]   in [65, N] psum -> DMA to SBUF ->
             PE-transpose back -> y = OUT*rcol + xr -> LayerNorm (bn_stats)
    out    bf16, converted to f32 on host.

Sharding: data-parallel over batch, 4 batches per core on 8 cores.
"""

import math

import numpy as np
import ml_dtypes

import concourse.bacc as bacc
import concourse.bass as bass
import concourse.tile as tile
from concourse import bass_utils, mybir

AF = mybir.ActivationFunctionType
ALU = mybir.AluOpType
DT = mybir.dt

B, N, D = 32, 1024, 64
NCORES = 8
BPC = B // NCORES          # batches per core
NT = N // 128              # 128-row tiles per batch

SCHRAUD_C4 = 128.0 / math.log(2.0)          # scale absorbed into W' (4th root)
SCHRAUD_B = 16256.0 - 128.0 * 0.0430        # bf16 exponent bias, sigma-centered

# of the 8 per-batch [128,1024] squares, how many run on ACT (rest on DVE)
K_ACT_SQ = 8


def _build_orthogonal(phi: np.ndarray, d: int = D) -> np.ndarray:
    """fp32 replica of the reference jax.lax.scan Givens chain."""
    O = np.eye(d, dtype=np.float32)
    ii, jj = np.triu_indices(d, k=1)
    c = np.cos(phi.astype(np.float32))
    s = np.sin(phi.astype(np.float32))
    for k in range(len(phi)):
        i, j = int(ii[k]), int(jj[k])
        ri = O[i].copy()
        rj = O[j].copy()
        O[i] = c[k] * ri + s[k] * rj
        O[j] = -s[k] * ri + c[k] * rj
    return O


def _build_nc(apply_gamma_beta: bool):
    nc = bacc.Bacc("TRN2", target_bir_lowering=False)

    wt_t = nc.dram_tensor("wt", [BPC, D, N], DT.float16, kind="ExternalInput")
    gt_t = nc.dram_tensor("gt", [BPC, D, N], DT.float16, kind="ExternalInput")
    xo_t = nc.dram_tensor("xo", [BPC, D, N], DT.float16, kind="ExternalInput")
    xr_t = nc.dram_tensor("xr", [BPC, N, D], DT.bfloat16, kind="ExternalInput")
    xs_t = nc.dram_tensor("xs", [BPC, N], DT.float32, kind="ExternalInput")
    vw_t = nc.dram_tensor("vw", [D, D + 1], DT.float16, kind="ExternalInput")
    id_t = nc.dram_tensor("ident", [128, 128], DT.float32, kind="ExternalInput")
    gb_t = nc.dram_tensor("gb", [2, D], DT.float32, kind="ExternalInput")
    out_t = nc.dram_tensor("out", [BPC, N, D], DT.bfloat16, kind="ExternalOutput")

    with tile.TileContext(nc) as tc:
        with (
            tc.tile_pool(name="const", bufs=1) as constp,
            tc.tile_pool(name="loadp", bufs=2) as loadp,
            tc.tile_pool(name="asqp", bufs=3) as asqp,
            tc.tile_pool(name="ep", bufs=3) as ep,
            tc.tile_pool(name="vp", bufs=2) as vp,
            tc.tile_pool(name="otp", bufs=2) as otp,
            tc.tile_pool(name="yp", bufs=2) as yp,
            tc.tile_pool(name="outp", bufs=2) as outp,
            tc.tile_pool(name="statp", bufs=2) as statp,
            tc.tile_pool(name="psA", bufs=3, space="PSUM") as psA,
            tc.tile_pool(name="psB", bufs=1, space="PSUM") as psB,
        ):
            vwb_sb = constp.tile([128, D + 1], DT.float16)
            nc.sync.dma_start(out=vwb_sb[0:D, :], in_=vw_t[:, :])
            nc.sync.dma_start(out=vwb_sb[D : 2 * D, :], in_=vw_t[:, :])
            id_sb = constp.tile([128, 128], DT.float32)
            nc.sync.dma_start(out=id_sb, in_=id_t[:, :])
            if apply_gamma_beta:
                gam_sb = constp.tile([128, D], DT.float32)
                nc.sync.dma_start(out=gam_sb, in_=gb_t[0, :].to_broadcast([128, D]))
                bet_sb = constp.tile([128, D], DT.float32)
                nc.sync.dma_start(out=bet_sb, in_=gb_t[1, :].to_broadcast([128, D]))

            # PE warm-up: dependency-free matmuls trip the HAM activity
            # window so real matmuls run at 2.4 GHz, not 1.2.
            warm = constp.tile([64, 512], DT.float16)
            nc.vector.memset(warm, 0.0)
            pw = psA.tile([128, 512], DT.float32, tag="pr", name="pw")
            for _ in range(8):
                nc.tensor.matmul(pw[0:64, :], lhsT=warm[:, 0:64], rhs=warm)

            def emit_loads(b):
                wt = loadp.tile([128, N], DT.float16, tag="wt")
                nc.sync.dma_start(out=wt[0:D, :], in_=wt_t[b])
                nc.sync.dma_start(out=wt[D : 2 * D, :], in_=wt_t[b])
                gt = loadp.tile([128, N], DT.float16, tag="gt")
                nc.sync.dma_start(out=gt[0:D, :], in_=gt_t[b])
                nc.sync.dma_start(out=gt[D : 2 * D, :], in_=gt_t[b])
                xo = loadp.tile([128, N], DT.float16, tag="xo")
                nc.sync.dma_start(out=xo[0:D, :], in_=xo_t[b])
                nc.sync.dma_start(out=xo[D : 2 * D, :], in_=xo_t[b])
                xr = loadp.tile([128, NT, D], DT.bfloat16, tag="xr")
                nc.sync.dma_start(
                    out=xr, in_=xr_t[b].rearrange("(t p) d -> p t d", p=128)
                )
                xs = loadp.tile([128, NT], DT.float32, tag="xs")
                nc.sync.dma_start(
                    out=xs, in_=xs_t[b].rearrange("(t p) -> p t", p=128)
                )
                return wt, gt, xo, xr, xs

            def emit_v(xo):
                # Vt = [X Vw^T | X vw_rowsum] (bias folded into residual on
                # host; the extra column yields sum_d num for the LN mean);
                # 64-contract pairs run concurrently in the two PE row groups.
                # Per-tile stride padded to 128 floats so no matmul output
                # crosses a PSUM bank boundary.
                # NB: tile_position=(64,0) matmuls fault at runtime unless the
                # PSUM output is bank-aligned, so these all run on group (0,0).
                pv = psA.tile([128, NT, 128], DT.float32, tag="pr", name="pv")
                for t in range(NT):
                    nc.tensor.matmul(
                        pv[:, t, 0 : D + 1],
                        lhsT=xo[0:D, t * 128 : (t + 1) * 128],
                        rhs=vwb_sb[0:D, :],
                        tile_position=(0, 0),
                    )
                v_sb = vp.tile([128, NT, 66], DT.bfloat16, tag="v")
                nc.vector.tensor_copy(v_sb[:, :, 0 : D + 1], pv[:, :, 0 : D + 1])
                nc.vector.memset(v_sb[:, :, D + 1], 1.0)
                return v_sb

            state = {0: emit_loads(0)}
            vg = {0: emit_v(state[0][2])}

            for b in range(BPC):
                wt, gt, xo, xr, xs = state[b]
                v_sb = vg[b]

                pot = psB.tile([128, N], DT.float32, tag="pot", name="pot")

                # ---- per pair of j-tiles: R (both PE row groups), then per
                # tile: square, schraudolph-exp, PV accumulate ----
                for q in range(4):
                    je, jo = 2 * q, 2 * q + 1
                    pre = psA.tile([128, N], DT.float32, tag="pr", name="pre")
                    pro = psA.tile([128, N], DT.float32, tag="pr", name="pro")
                    for c in range(2):
                        cs = slice(c * 512, (c + 1) * 512)
                        nc.tensor.matmul(
                            pre[:, cs],
                            lhsT=wt[0:D, je * 128 : (je + 1) * 128],
                            rhs=gt[0:D, cs],
                            tile_position=(0, 0),
                        )
                    for c in range(2):
                        cs = slice(c * 512, (c + 1) * 512)
                        nc.tensor.matmul(
                            pro[:, cs],
                            lhsT=wt[D : 2 * D, jo * 128 : (jo + 1) * 128],
                            rhs=gt[D : 2 * D, cs],
                            tile_position=(64, 0),
                        )
                    for jt, pr in ((je, pre), (jo, pro)):
                        asq = asqp.tile([128, N], DT.float16, tag="asq")
                        if jt < K_ACT_SQ:
                            nc.scalar.activation(asq, pr, AF.Square)
                        else:
                            nc.vector.tensor_tensor(
                                out=asq, in0=pr, in1=pr, op=ALU.mult
                            )
                        e = ep.tile([128, N], DT.uint16, tag="e")
                        nc.vector.tensor_scalar(
                            e, asq, scalar1=SCHRAUD_B, scalar2=None, op0=ALU.add
                        )
                        eb = e.bitcast(DT.bfloat16)
                        for c in range(2):
                            nc.tensor.matmul(
                                pot[0 : D + 2, c * 512 : (c + 1) * 512],
                                lhsT=v_sb[:, jt, 0 : D + 2],
                                rhs=eb[:, c * 512 : (c + 1) * 512],
                                start=(jt == 0),
                                stop=(jt == NT - 1),
                            )
                    if q == 1 and b + 1 < BPC:
                        state[b + 1] = emit_loads(b + 1)
                    if q == 2 and b + 1 < BPC:
                        vg[b + 1] = emit_v(state[b + 1][2])

                # ---- OUT^T -> SBUF (half DVE, half ACT) ----
                ot = otp.tile([128, N], DT.float32, tag="ot")
                nc.vector.tensor_copy(ot[0 : D + 2, 0:512], pot[0 : D + 2, 0:512])
                nc.scalar.copy(ot[0 : D + 2, 512:N], pot[0 : D + 2, 512:N])

                # ---- transpose back; y = OUT*(1/rowsum) + xr ----
                # ptr cols: 0..63 num, 64 sum_d num, 65 denominator
                rcol = statp.tile([128, NT], DT.float32, tag="rcol")
                mus = statp.tile([128, NT], DT.float32, tag="mus")
                y = yp.tile([128, NT, D], DT.bfloat16, tag="y")
                for grp in range(2):
                    ptr = psA.tile([128, 4, D + 2], DT.float32, tag="pr", name="ptr")
                    for qq in range(4):
                        it = grp * 4 + qq
                        nc.tensor.transpose(
                            ptr[:, qq, :],
                            ot[0 : D + 2, it * 128 : (it + 1) * 128],
                            id_sb[0 : D + 2, 0 : D + 2],
                        )
                    g_sl = slice(grp * 4, grp * 4 + 4)
                    nc.vector.reciprocal_approx_fast(rcol[:, g_sl], ptr[:, :, D + 1])
                    rc4 = rcol[:, g_sl]
                    rcol_bc = bass.AP(
                        tensor=rcol.tensor, offset=rc4.offset,
                        ap=[rcol.ap[0], [1, 4], [0, D]],
                    )
                    nc.vector.tensor_tensor(
                        out=y[:, g_sl, :], in0=ptr[:, :, 0:D], in1=rcol_bc,
                        op=ALU.mult,
                    )
                    nc.vector.tensor_tensor(
                        out=mus[:, g_sl], in0=ptr[:, :, D], in1=rc4, op=ALU.mult
                    )
                nc.vector.tensor_add(y, y, xr)

                # ---- LayerNorm stats: mean from the free sum_d num column,
                # variance from one squared reduce ----
                ysq = yp.tile([128, NT, D], DT.bfloat16, tag="ysq")
                nc.vector.tensor_mul(ysq, y, y)
                ys2 = statp.tile([128, NT], DT.float32, tag="ys2")
                nc.vector.reduce_sum(ys2, ysq, axis=mybir.AxisListType.X)
                mu = statp.tile([128, NT], DT.float32, tag="mu")
                nc.gpsimd.tensor_scalar(
                    mu, mus, scalar1=1.0 / D, scalar2=None, op0=ALU.mult
                )
                nc.gpsimd.tensor_add(mu, mu, xs)
                # var + eps = ys2/64 - mu^2 + 1e-5
                m2 = statp.tile([128, NT], DT.float32, tag="m2")
                nc.gpsimd.tensor_mul(m2, mu, mu)
                ve = statp.tile([128, NT], DT.float32, tag="ve")
                nc.gpsimd.tensor_scalar(
                    ve, ys2, scalar1=1.0 / D, scalar2=1e-5, op0=ALU.mult, op1=ALU.add
                )
                nc.gpsimd.tensor_sub(ve, ve, m2)
                sd = statp.tile([128, NT], DT.float32, tag="sd")
                nc.scalar.activation(sd, ve, AF.Sqrt)
                rstd = statp.tile([128, NT], DT.float32, tag="rstd")
                nc.vector.reciprocal_approx_fast(rstd, sd)

                # ---- normalize per i-tile on Pool: out = (y - mu) * rstd ----
                out_sb = outp.tile([128, NT, D], DT.bfloat16, tag="o")
                for t in range(NT):
                    nc.vector.tensor_scalar(
                        out_sb[:, t, :], y[:, t, :],
                        scalar1=mu[:, t : t + 1], scalar2=rstd[:, t : t + 1],
                        op0=ALU.subtract, op1=ALU.mult,
                    )
                if apply_gamma_beta:
                    for t in range(NT):
                        nc.gpsimd.tensor_mul(out_sb[:, t, :], out_sb[:, t, :], gam_sb)
                        nc.gpsimd.tensor_add(out_sb[:, t, :], out_sb[:, t, :], bet_sb)

                nc.sync.dma_start(
                    out=out_t[b].rearrange("(t p) d -> p t d", p=128), in_=out_sb
                )

    nc.compile()
    return nc


_NC_CACHE: dict = {}


def kernel(input1, V_w, V_b, phi, ln_gamma, ln_beta, _trace=False):
    input1 = np.ascontiguousarray(np.asarray(input1, dtype=np.float32))
    V_w = np.asarray(V_w, dtype=np.float32)
    V_b = np.asarray(V_b, dtype=np.float32)
    phi = np.asarray(phi, dtype=np.float32)
    ln_gamma = np.asarray(ln_gamma, dtype=np.float32)
    ln_beta = np.asarray(ln_beta, dtype=np.float32)

    apply_gb = not (np.all(ln_gamma == 1.0) and np.all(ln_beta == 0.0))

    if apply_gb not in _NC_CACHE:
        _NC_CACHE[apply_gb] = _build_nc(apply_gb)
    nc = _NC_CACHE[apply_gb]

    O = _build_orthogonal(phi)
    vwT = V_w.T.astype(np.float32)
    vw16 = np.ascontiguousarray(
        np.concatenate([vwT, vwT.sum(axis=1, keepdims=True)], axis=1)
        .astype(np.float16)
    )
    ident = np.eye(128, dtype=np.float32)
    gb = np.ascontiguousarray(np.stack([ln_gamma, ln_beta]).astype(np.float32))

    # host prep: W' = x * ||x||^-1/2 * 8^-1/4 * (128/ln2)^1/4 (transposed f16),
    # gt = (W' O)^T, xo = X^T, xr = x + V_b (bf16 residual w/ folded bias)
    ss = (input1.astype(np.float64) ** 2).sum(-1)
    s = (ss ** -0.25 * 8.0 ** -0.25 * SCHRAUD_C4 ** 0.25).astype(np.float32)
    w = input1 * s[..., None]
    wt_full = np.ascontiguousarray(w.transpose(0, 2, 1).astype(np.float16))
    gt_full = np.ascontiguousarray(
        (w @ O).transpose(0, 2, 1).astype(np.float16)
    )
    xo_full = np.ascontiguousarray(
        input1.transpose(0, 2, 1).astype(np.float16)
    )
    xr_f32 = input1 + V_b
    xr_full = np.ascontiguousarray(xr_f32.astype(ml_dtypes.bfloat16))
    xs_full = np.ascontiguousarray((xr_f32.sum(-1) / D).astype(np.float32))

    in_maps = []
    for c in range(NCORES):
        sl = slice(c * BPC, (c + 1) * BPC)
        in_maps.append(
            {
                "wt": wt_full[sl],
                "gt": gt_full[sl],
                "xo": xo_full[sl],
                "xr": xr_full[sl],
                "xs": xs_full[sl],
                "vw": vw16,
                "ident": ident,
                "gb": gb,
            }
        )

    res = bass_utils.run_bass_kernel_spmd(
        nc, in_maps, core_ids=list(range(NCORES)), trace=_trace
    )
    out = np.concatenate(
        [res.results[c]["out"].astype(np.float32) for c in range(NCORES)], axis=0
    )
    if _trace:
        kernel._last_result = res
    return out


# revision 23
# speedup vs baseline: 1.2581x; 1.1334x over previous
"""Trainium2 Bass kernel for the hybrid attention head (nn_AttentionHead_Hybrid).

Math (per batch):
    norms  n_i = ||x_i||;  xh = x / n
    O      = product of 2016 Givens rotations (built on host, fp32)
    S[i,j] = xh_i . O . xh_j
    A      = S^2 * n_i n_j ;  P = softmax(A / 8)
    V      = x @ Vw^T + Vb
    out    = LayerNorm(P @ V + x) * gamma + beta

Device formulation (per core, 4 batches):
    W'     = diag(s') X with s'_n = ||x_n||^-1/2 * 8^-1/4 * (128/ln2)^1/4
    gt     = (W' O)^T  (host-prepped, f16)          so R[j,i] = w'_j O^T w'_i
    R^2    = (128/ln2) * A/8
    E      = exp(A/8) via Schraudolph: bitcast_bf16(int16(R^2 + B))  [one DVE
             tensor_scalar in 4x mode; the sawtooth error cancels in softmax]
    Vt     = [X Vw^T | 1]  (bf16; ones column gives softmax row-sums for free;
             V_b folded into the residual on host: xr = x + V_b)
    OUT^T  = sum_j Vt[j,:]^T E^T[j,:]   in [65, N] psum -> DMA to SBUF ->
             PE-transpose back -> y = OUT*rcol + xr -> LayerNorm (bn_stats)
    out    bf16, converted to f32 on host.

Sharding: data-parallel over batch, 4 batches per core on 8 cores.
"""

import math

import numpy as np
import ml_dtypes

import concourse.bacc as bacc
import concourse.bass as bass
import concourse.tile as tile
from concourse import bass_utils, mybir

AF = mybir.ActivationFunctionType
ALU = mybir.AluOpType
DT = mybir.dt

B, N, D = 32, 1024, 64
NCORES = 8
BPC = B // NCORES          # batches per core
NT = N // 128              # 128-row tiles per batch

SCHRAUD_C4 = 128.0 / math.log(2.0)          # scale absorbed into W' (4th root)
SCHRAUD_B = 16256.0 - 128.0 * 0.0430        # bf16 exponent bias, sigma-centered

# of the 8 per-batch [128,1024] squares, how many run on ACT (rest on DVE)
K_ACT_SQ = 8


def _build_orthogonal(phi: np.ndarray, d: int = D) -> np.ndarray:
    """fp32 replica of the reference jax.lax.scan Givens chain."""
    O = np.eye(d, dtype=np.float32)
    ii, jj = np.triu_indices(d, k=1)
    c = np.cos(phi.astype(np.float32))
    s = np.sin(phi.astype(np.float32))
    for k in range(len(phi)):
        i, j = int(ii[k]), int(jj[k])
        ri = O[i].copy()
        rj = O[j].copy()
        O[i] = c[k] * ri + s[k] * rj
        O[j] = -s[k] * ri + c[k] * rj
    return O


def _build_nc(apply_gamma_beta: bool):
    nc = bacc.Bacc("TRN2", target_bir_lowering=False)

    wt_t = nc.dram_tensor("wt", [BPC, D, N], DT.float16, kind="ExternalInput")
    gt_t = nc.dram_tensor("gt", [BPC, D, N], DT.float16, kind="ExternalInput")
    xo_t = nc.dram_tensor("xo", [BPC, D, N], DT.float16, kind="ExternalInput")
    xr_t = nc.dram_tensor("xr", [BPC, N, D], DT.bfloat16, kind="ExternalInput")
    xs_t = nc.dram_tensor("xs", [BPC, N], DT.float32, kind="ExternalInput")
    vw_t = nc.dram_tensor("vw", [D, D + 1], DT.float16, kind="ExternalInput")
    id_t = nc.dram_tensor("ident", [128, 128], DT.float32, kind="ExternalInput")
    gb_t = nc.dram_tensor("gb", [2, D], DT.float32, kind="ExternalInput")
    out_t = nc.dram_tensor("out", [BPC, N, D], DT.bfloat16, kind="ExternalOutput")

    with tile.TileContext(nc) as tc:
        with (
            tc.tile_pool(name="const", bufs=1) as constp,
            tc.tile_pool(name="loadp", bufs=2) as loadp,
            tc.tile_pool(name="asqp", bufs=3) as asqp,
            tc.tile_pool(name="ep", bufs=3) as ep,
            tc.tile_pool(name="vp", bufs=2) as vp,
            tc.tile_pool(name="otp", bufs=2) as otp,
            tc.tile_pool(name="yp", bufs=2) as yp,
            tc.tile_pool(name="outp", bufs=2) as outp,
            tc.tile_pool(name="statp", bufs=2) as statp,
            tc.tile_pool(name="psA", bufs=3, space="PSUM") as psA,
            tc.tile_pool(name="psB", bufs=1, space="PSUM") as psB,
        ):
            vwb_sb = constp.tile([128, D + 1], DT.float16)
            nc.sync.dma_start(out=vwb_sb[0:D, :], in_=vw_t[:, :])
            nc.sync.dma_start(out=vwb_sb[D : 2 * D, :], in_=vw_t[:, :])
            id_sb = constp.tile([128, 128], DT.float32)
            nc.sync.dma_start(out=id_sb, in_=id_t[:, :])
            if apply_gamma_beta:
                gam_sb = constp.tile([128, D], DT.float32)
                nc.sync.dma_start(out=gam_sb, in_=gb_t[0, :].to_broadcast([128, D]))
                bet_sb = constp.tile([128, D], DT.float32)
                nc.sync.dma_start(out=bet_sb, in_=gb_t[1, :].to_broadcast([128, D]))

            # PE warm-up: dependency-free matmuls trip the HAM activity
            # window so real matmuls run at 2.4 GHz, not 1.2.
            warm = constp.tile([64, 512], DT.float16)
            nc.vector.memset(warm, 0.0)
            pw = psA.tile([128, 512], DT.float32, tag="pr", name="pw")
            for _ in range(8):
                nc.tensor.matmul(pw[0:64, :], lhsT=warm[:, 0:64], rhs=warm)

            def emit_loads(b):
                wt = loadp.tile([128, N], DT.float16, tag="wt")
                nc.sync.dma_start(out=wt[0:D, :], in_=wt_t[b])
                nc.sync.dma_start(out=wt[D : 2 * D, :], in_=wt_t[b])
                gt = loadp.tile([128, N], DT.float16, tag="gt")
                nc.sync.dma_start(out=gt[0:D, :], in_=gt_t[b])
                nc.sync.dma_start(out=gt[D : 2 * D, :], in_=gt_t[b])
                xo = loadp.tile([128, N], DT.float16, tag="xo")
                nc.sync.dma_start(out=xo[0:D, :], in_=xo_t[b])
                nc.sync.dma_start(out=xo[D : 2 * D, :], in_=xo_t[b])
                xr = loadp.tile([128, NT, D], DT.bfloat16, tag="xr")
                nc.sync.dma_start(
                    out=xr, in_=xr_t[b].rearrange("(t p) d -> p t d", p=128)
                )
                xs = loadp.tile([128, NT], DT.float32, tag="xs")
                nc.sync.dma_start(
                    out=xs, in_=xs_t[b].rearrange("(t p) -> p t", p=128)
                )
                return wt, gt, xo, xr, xs

            def emit_v(xo):
                # Vt = [X Vw^T | X vw_rowsum] (bias folded into residual on
                # host; the extra column yields sum_d num for the LN mean);
                # 64-contract pairs run concurrently in the two PE row groups.
                # Per-tile stride padded to 128 floats so no matmul output
                # crosses a PSUM bank boundary.
                # NB: tile_position=(64,0) matmuls fault at runtime unless the
                # PSUM output is bank-aligned, so these all run on group (0,0).
                pv = psA.tile([128, NT, 128], DT.float32, tag="pr", name="pv")
                for t in range(NT):
                    nc.tensor.matmul(
                        pv[:, t, 0 : D + 1],
                        lhsT=xo[0:D, t * 128 : (t + 1) * 128],
                        rhs=vwb_sb[0:D, :],
                        tile_position=(0, 0),
                    )
                v_sb = vp.tile([128, NT, 66], DT.bfloat16, tag="v")
                nc.vector.tensor_copy(v_sb[:, :, 0 : D + 1], pv[:, :, 0 : D + 1])
                nc.vector.memset(v_sb[:, :, D + 1], 1.0)
                return v_sb

            state = {0: emit_loads(0)}
            vg = {0: emit_v(state[0][2])}

            for b in range(BPC):
                wt, gt, xo, xr, xs = state[b]
                v_sb = vg[b]

                pot = psB.tile([128, N], DT.float32, tag="pot", name="pot")

                # ---- per j-tile: R via both PE row groups on disjoint column
                # halves of the same tile (adjacent emission -> they overlap),
                # then square, schraudolph-exp, PV accumulate ----
                for jt in range(NT):
                    pr = psA.tile([128, N], DT.float32, tag="pr", name="pr")
                    nc.tensor.matmul(
                        pr[:, 0:512],
                        lhsT=wt[0:D, jt * 128 : (jt + 1) * 128],
                        rhs=gt[0:D, 0:512],
                        tile_position=(0, 0),
                    )
                    nc.tensor.matmul(
                        pr[:, 512:N],
                        lhsT=wt[D : 2 * D, jt * 128 : (jt + 1) * 128],
                        rhs=gt[D : 2 * D, 512:N],
                        tile_position=(64, 0),
                    )
                    asq = asqp.tile([128, N], DT.float16, tag="asq")
                    if jt < K_ACT_SQ:
                        nc.scalar.activation(asq, pr, AF.Square)
                    else:
                        nc.vector.tensor_tensor(
                            out=asq, in0=pr, in1=pr, op=ALU.mult
                        )
                    e = ep.tile([128, N], DT.uint16, tag="e")
                    nc.vector.tensor_scalar(
                        e, asq, scalar1=SCHRAUD_B, scalar2=None, op0=ALU.add
                    )
                    eb = e.bitcast(DT.bfloat16)
                    for c in range(2):
                        nc.tensor.matmul(
                            pot[0 : D + 2, c * 512 : (c + 1) * 512],
                            lhsT=v_sb[:, jt, 0 : D + 2],
                            rhs=eb[:, c * 512 : (c + 1) * 512],
                            start=(jt == 0),
                            stop=(jt == NT - 1),
                        )
                    if jt == 2 and b + 1 < BPC:
                        state[b + 1] = emit_loads(b + 1)
                    if jt == 4 and b + 1 < BPC:
                        vg[b + 1] = emit_v(state[b + 1][2])

                # ---- OUT^T -> SBUF (half DVE, half ACT) ----
                ot = otp.tile([128, N], DT.float32, tag="ot")
                nc.vector.tensor_copy(ot[0 : D + 2, 0:512], pot[0 : D + 2, 0:512])
                nc.scalar.copy(ot[0 : D + 2, 512:N], pot[0 : D + 2, 512:N])

                # ---- transpose back; y = OUT*(1/rowsum) + xr ----
                # ptr cols: 0..63 num, 64 sum_d num, 65 denominator
                rcol = statp.tile([128, NT], DT.float32, tag="rcol")
                mus = statp.tile([128, NT], DT.float32, tag="mus")
                y = yp.tile([128, NT, D], DT.bfloat16, tag="y")
                for grp in range(2):
                    ptr = psA.tile([128, 4, D + 2], DT.float32, tag="pr", name="ptr")
                    for qq in range(4):
                        it = grp * 4 + qq
                        nc.tensor.transpose(
                            ptr[:, qq, :],
                            ot[0 : D + 2, it * 128 : (it + 1) * 128],
                            id_sb[0 : D + 2, 0 : D + 2],
                        )
                    g_sl = slice(grp * 4, grp * 4 + 4)
                    nc.vector.reciprocal_approx_fast(rcol[:, g_sl], ptr[:, :, D + 1])
                    rc4 = rcol[:, g_sl]
                    rcol_bc = bass.AP(
                        tensor=rcol.tensor, offset=rc4.offset,
                        ap=[rcol.ap[0], [1, 4], [0, D]],
                    )
                    nc.vector.tensor_tensor(
                        out=y[:, g_sl, :], in0=ptr[:, :, 0:D], in1=rcol_bc,
                        op=ALU.mult,
                    )
                    nc.vector.tensor_tensor(
                        out=mus[:, g_sl], in0=ptr[:, :, D], in1=rc4, op=ALU.mult
                    )
                nc.vector.tensor_add(y, y, xr)

                # ---- LayerNorm stats: mean from the free sum_d num column,
                # variance from one squared reduce ----
                ysq = yp.tile([128, NT, D], DT.bfloat16, tag="ysq")
                nc.vector.tensor_mul(ysq, y, y)
                ys2 = statp.tile([128, NT], DT.float32, tag="ys2")
                nc.vector.reduce_sum(ys2, ysq, axis=mybir.AxisListType.X)
                mu = statp.tile([128, NT], DT.float32, tag="mu")
                nc.gpsimd.tensor_scalar(
                    mu, mus, scalar1=1.0 / D, scalar2=None, op0=ALU.mult
                )
                nc.gpsimd.tensor_add(mu, mu, xs)
                # var + eps = ys2/64 - mu^2 + 1e-5
                m2 = statp.tile([128, NT], DT.float32, tag="m2")
                nc.gpsimd.tensor_mul(m2, mu, mu)
                ve = statp.tile([128, NT], DT.float32, tag="ve")
                nc.gpsimd.tensor_scalar(
                    ve, ys2, scalar1=1.0 / D, scalar2=1e-5, op0=ALU.mult, op1=ALU.add
                )
                nc.gpsimd.tensor_sub(ve, ve, m2)
                sd = statp.tile([128, NT], DT.float32, tag="sd")
                nc.scalar.activation(sd, ve, AF.Sqrt)
                rstd = statp.tile([128, NT], DT.float32, tag="rstd")
                nc.vector.reciprocal_approx_fast(rstd, sd)

                # ---- normalize per i-tile on Pool: out = (y - mu) * rstd ----
                out_sb = outp.tile([128, NT, D], DT.bfloat16, tag="o")
                for t in range(NT):
                    nc.vector.tensor_scalar(
                        out_sb[:, t, :], y[:, t, :],
                        scalar1=mu[:, t : t + 1], scalar2=rstd[:, t : t + 1],
                        op0=ALU.subtract, op1=ALU.mult,
                    )
                if apply_gamma_beta:
                    for t in range(NT):
                        nc.gpsimd.tensor_mul(out_sb[:, t, :], out_sb[:, t, :], gam_sb)
                        nc.gpsimd.tensor_add(out_sb[:, t, :], out_sb[:, t, :], bet_sb)

                nc.sync.dma_start(
                    out=out_t[b].rearrange("(t p) d -> p t d", p=128), in_=out_sb
                )

    nc.compile()
    return nc


_NC_CACHE: dict = {}


def kernel(input1, V_w, V_b, phi, ln_gamma, ln_beta, _trace=False):
    input1 = np.ascontiguousarray(np.asarray(input1, dtype=np.float32))
    V_w = np.asarray(V_w, dtype=np.float32)
    V_b = np.asarray(V_b, dtype=np.float32)
    phi = np.asarray(phi, dtype=np.float32)
    ln_gamma = np.asarray(ln_gamma, dtype=np.float32)
    ln_beta = np.asarray(ln_beta, dtype=np.float32)

    apply_gb = not (np.all(ln_gamma == 1.0) and np.all(ln_beta == 0.0))

    if apply_gb not in _NC_CACHE:
        _NC_CACHE[apply_gb] = _build_nc(apply_gb)
    nc = _NC_CACHE[apply_gb]

    O = _build_orthogonal(phi)
    vwT = V_w.T.astype(np.float32)
    vw16 = np.ascontiguousarray(
        np.concatenate([vwT, vwT.sum(axis=1, keepdims=True)], axis=1)
        .astype(np.float16)
    )
    ident = np.eye(128, dtype=np.float32)
    gb = np.ascontiguousarray(np.stack([ln_gamma, ln_beta]).astype(np.float32))

    # host prep: W' = x * ||x||^-1/2 * 8^-1/4 * (128/ln2)^1/4 (transposed f16),
    # gt = (W' O)^T, xo = X^T, xr = x + V_b (bf16 residual w/ folded bias)
    ss = (input1.astype(np.float64) ** 2).sum(-1)
    s = (ss ** -0.25 * 8.0 ** -0.25 * SCHRAUD_C4 ** 0.25).astype(np.float32)
    w = input1 * s[..., None]
    wt_full = np.ascontiguousarray(w.transpose(0, 2, 1).astype(np.float16))
    gt_full = np.ascontiguousarray(
        (w @ O).transpose(0, 2, 1).astype(np.float16)
    )
    xo_full = np.ascontiguousarray(
        input1.transpose(0, 2, 1).astype(np.float16)
    )
    xr_f32 = input1 + V_b
    xr_full = np.ascontiguousarray(xr_f32.astype(ml_dtypes.bfloat16))
    xs_full = np.ascontiguousarray((xr_f32.sum(-1) / D).astype(np.float32))

    in_maps = []
    for c in range(NCORES):
        sl = slice(c * BPC, (c + 1) * BPC)
        in_maps.append(
            {
                "wt": wt_full[sl],
                "gt": gt_full[sl],
                "xo": xo_full[sl],
                "xr": xr_full[sl],
                "xs": xs_full[sl],
                "vw": vw16,
                "ident": ident,
                "gb": gb,
            }
        )

    res = bass_utils.run_bass_kernel_spmd(
        nc, in_maps, core_ids=list(range(NCORES)), trace=_trace
    )
    out = np.concatenate(
        [res.results[c]["out"].astype(np.float32) for c in range(NCORES)], axis=0
    )
    if _trace:
        kernel._last_result = res
    return out


# revision 24
# speedup vs baseline: 1.2794x; 1.0169x over previous
"""Trainium2 Bass kernel for the hybrid attention head (nn_AttentionHead_Hybrid).

Math (per batch):
    norms  n_i = ||x_i||;  xh = x / n
    O      = product of 2016 Givens rotations (built on host, fp32)
    S[i,j] = xh_i . O . xh_j
    A      = S^2 * n_i n_j ;  P = softmax(A / 8)
    V      = x @ Vw^T + Vb
    out    = LayerNorm(P @ V + x) * gamma + beta

Device formulation (per core, 4 batches):
    W'     = diag(s') X with s'_n = ||x_n||^-1/2 * 8^-1/4 * (128/ln2)^1/4
    gt     = (W' O)^T  (host-prepped, f16)          so R[j,i] = w'_j O^T w'_i
    R^2    = (128/ln2) * A/8
    E      = exp(A/8) via Schraudolph: bitcast_bf16(int16(R^2 + B))  [one DVE
             tensor_scalar in 4x mode; the sawtooth error cancels in softmax]
    Vt     = [X Vw^T | 1]  (bf16; ones column gives softmax row-sums for free;
             V_b folded into the residual on host: xr = x + V_b)
    OUT^T  = sum_j Vt[j,:]^T E^T[j,:]   in [65, N] psum -> DMA to SBUF ->
             PE-transpose back -> y = OUT*rcol + xr -> LayerNorm (bn_stats)
    out    bf16, converted to f32 on host.

Sharding: data-parallel over batch, 4 batches per core on 8 cores.
"""

import math

import numpy as np
import ml_dtypes

import concourse.bacc as bacc
import concourse.bass as bass
import concourse.tile as tile
from concourse import bass_utils, mybir

AF = mybir.ActivationFunctionType
ALU = mybir.AluOpType
DT = mybir.dt

B, N, D = 32, 1024, 64
NCORES = 8
BPC = B // NCORES          # batches per core
NT = N // 128              # 128-row tiles per batch

SCHRAUD_C4 = 128.0 / math.log(2.0)          # scale absorbed into W' (4th root)
SCHRAUD_B = 16256.0 - 128.0 * 0.0430        # bf16 exponent bias, sigma-centered

# of the 8 per-batch [128,1024] squares, how many run on ACT (rest on DVE)
K_ACT_SQ = 8


def _build_orthogonal(phi: np.ndarray, d: int = D) -> np.ndarray:
    """fp32 replica of the reference jax.lax.scan Givens chain."""
    O = np.eye(d, dtype=np.float32)
    ii, jj = np.triu_indices(d, k=1)
    c = np.cos(phi.astype(np.float32))
    s = np.sin(phi.astype(np.float32))
    for k in range(len(phi)):
        i, j = int(ii[k]), int(jj[k])
        ri = O[i].copy()
        rj = O[j].copy()
        O[i] = c[k] * ri + s[k] * rj
        O[j] = -s[k] * ri + c[k] * rj
    return O


def _build_nc(apply_gamma_beta: bool):
    nc = bacc.Bacc("TRN2", target_bir_lowering=False)

    wt_t = nc.dram_tensor("wt", [BPC, D, N], DT.float16, kind="ExternalInput")
    gt_t = nc.dram_tensor("gt", [BPC, D, N], DT.float16, kind="ExternalInput")
    xo_t = nc.dram_tensor("xo", [BPC, D, N], DT.float16, kind="ExternalInput")
    xr_t = nc.dram_tensor("xr", [BPC, N, D], DT.bfloat16, kind="ExternalInput")
    xs_t = nc.dram_tensor("xs", [BPC, N], DT.float32, kind="ExternalInput")
    vw_t = nc.dram_tensor("vw", [D, D + 1], DT.float16, kind="ExternalInput")
    id_t = nc.dram_tensor("ident", [128, 128], DT.float32, kind="ExternalInput")
    gb_t = nc.dram_tensor("gb", [2, D], DT.float32, kind="ExternalInput")
    out_t = nc.dram_tensor("out", [BPC, N, D], DT.bfloat16, kind="ExternalOutput")

    with tile.TileContext(nc) as tc:
        with (
            tc.tile_pool(name="const", bufs=1) as constp,
            tc.tile_pool(name="loadp", bufs=2) as loadp,
            tc.tile_pool(name="asqp", bufs=3) as asqp,
            tc.tile_pool(name="ep", bufs=3) as ep,
            tc.tile_pool(name="vp", bufs=2) as vp,
            tc.tile_pool(name="otp", bufs=2) as otp,
            tc.tile_pool(name="yp", bufs=2) as yp,
            tc.tile_pool(name="outp", bufs=2) as outp,
            tc.tile_pool(name="statp", bufs=2) as statp,
            tc.tile_pool(name="psA", bufs=3, space="PSUM") as psA,
            tc.tile_pool(name="psB", bufs=1, space="PSUM") as psB,
        ):
            vwb_sb = constp.tile([128, D + 1], DT.float16)
            nc.sync.dma_start(out=vwb_sb[0:D, :], in_=vw_t[:, :])
            nc.sync.dma_start(out=vwb_sb[D : 2 * D, :], in_=vw_t[:, :])
            id_sb = constp.tile([128, 128], DT.float32)
            nc.sync.dma_start(out=id_sb, in_=id_t[:, :])
            if apply_gamma_beta:
                gam_sb = constp.tile([128, D], DT.float32)
                nc.sync.dma_start(out=gam_sb, in_=gb_t[0, :].to_broadcast([128, D]))
                bet_sb = constp.tile([128, D], DT.float32)
                nc.sync.dma_start(out=bet_sb, in_=gb_t[1, :].to_broadcast([128, D]))


            def emit_loads(b):
                wt = loadp.tile([128, N], DT.float16, tag="wt")
                nc.sync.dma_start(out=wt[0:D, :], in_=wt_t[b])
                nc.sync.dma_start(out=wt[D : 2 * D, :], in_=wt_t[b])
                gt = loadp.tile([128, N], DT.float16, tag="gt")
                nc.sync.dma_start(out=gt[0:D, :], in_=gt_t[b])
                nc.sync.dma_start(out=gt[D : 2 * D, :], in_=gt_t[b])
                xo = loadp.tile([128, N], DT.float16, tag="xo")
                nc.sync.dma_start(out=xo[0:D, :], in_=xo_t[b])
                nc.sync.dma_start(out=xo[D : 2 * D, :], in_=xo_t[b])
                xr = loadp.tile([128, NT, D], DT.bfloat16, tag="xr")
                nc.sync.dma_start(
                    out=xr, in_=xr_t[b].rearrange("(t p) d -> p t d", p=128)
                )
                xs = loadp.tile([128, NT], DT.float32, tag="xs")
                nc.sync.dma_start(
                    out=xs, in_=xs_t[b].rearrange("(t p) -> p t", p=128)
                )
                return wt, gt, xo, xr, xs

            def emit_v(xo):
                # Vt = [X Vw^T | X vw_rowsum] (bias folded into residual on
                # host; the extra column yields sum_d num for the LN mean);
                # 64-contract pairs run concurrently in the two PE row groups.
                # Per-tile stride padded to 128 floats so no matmul output
                # crosses a PSUM bank boundary.
                # NB: tile_position=(64,0) matmuls fault at runtime unless the
                # PSUM output is bank-aligned, so these all run on group (0,0).
                pv = psA.tile([128, NT, 128], DT.float32, tag="pr", name="pv")
                for t in range(NT):
                    nc.tensor.matmul(
                        pv[:, t, 0 : D + 1],
                        lhsT=xo[0:D, t * 128 : (t + 1) * 128],
                        rhs=vwb_sb[0:D, :],
                        tile_position=(0, 0),
                    )
                v_sb = vp.tile([128, NT, 66], DT.bfloat16, tag="v")
                nc.vector.tensor_copy(v_sb[:, :, 0 : D + 1], pv[:, :, 0 : D + 1])
                nc.vector.memset(v_sb[:, :, D + 1], 1.0)
                return v_sb

            state = {0: emit_loads(0)}
            vg = {0: emit_v(state[0][2])}

            for b in range(BPC):
                wt, gt, xo, xr, xs = state[b]
                v_sb = vg[b]

                pot = psB.tile([128, N], DT.float32, tag="pot", name="pot")
                ebs = {}

                # ---- per j-tile: R via both PE row groups on disjoint column
                # halves of the same tile (adjacent emission -> they overlap),
                # then square, schraudolph-exp, PV accumulate ----
                for jt in range(NT):
                    pr = psA.tile([128, N], DT.float32, tag="pr", name="pr")
                    nc.tensor.matmul(
                        pr[:, 0:512],
                        lhsT=wt[0:D, jt * 128 : (jt + 1) * 128],
                        rhs=gt[0:D, 0:512],
                        tile_position=(0, 0),
                    )
                    nc.tensor.matmul(
                        pr[:, 512:N],
                        lhsT=wt[D : 2 * D, jt * 128 : (jt + 1) * 128],
                        rhs=gt[D : 2 * D, 512:N],
                        tile_position=(64, 0),
                    )
                    asq = asqp.tile([128, N], DT.float16, tag="asq")
                    if jt < K_ACT_SQ:
                        nc.scalar.activation(asq, pr, AF.Square)
                    else:
                        nc.vector.tensor_tensor(
                            out=asq, in0=pr, in1=pr, op=ALU.mult
                        )
                    e = ep.tile([128, N], DT.uint16, tag="e")
                    nc.vector.tensor_scalar(
                        e, asq, scalar1=SCHRAUD_B, scalar2=None, op0=ALU.add
                    )
                    ebs[jt] = e.bitcast(DT.bfloat16)
                    for pj in ([jt - 1] if jt < NT - 1 else [jt - 1, jt]):
                        if pj < 0:
                            continue
                        for c in range(2):
                            nc.tensor.matmul(
                                pot[0 : D + 2, c * 512 : (c + 1) * 512],
                                lhsT=v_sb[:, pj, 0 : D + 2],
                                rhs=ebs[pj][:, c * 512 : (c + 1) * 512],
                                start=(pj == 0),
                                stop=(pj == NT - 1),
                            )
                    if jt == 2 and b + 1 < BPC:
                        state[b + 1] = emit_loads(b + 1)
                    if jt == 4 and b + 1 < BPC:
                        vg[b + 1] = emit_v(state[b + 1][2])

                # ---- OUT^T -> SBUF (half DVE, half ACT) ----
                ot = otp.tile([128, N], DT.float32, tag="ot")
                nc.vector.tensor_copy(ot[0 : D + 2, 0:512], pot[0 : D + 2, 0:512])
                nc.scalar.copy(ot[0 : D + 2, 512:N], pot[0 : D + 2, 512:N])

                # ---- transpose back; y = OUT*(1/rowsum) + xr ----
                # ptr cols: 0..63 num, 64 sum_d num, 65 denominator
                rcol = statp.tile([128, NT], DT.float32, tag="rcol")
                mus = statp.tile([128, NT], DT.float32, tag="mus")
                y = yp.tile([128, NT, D], DT.bfloat16, tag="y")
                for grp in range(2):
                    ptr = psA.tile([128, 4, D + 2], DT.float32, tag="pr", name="ptr")
                    for qq in range(4):
                        it = grp * 4 + qq
                        nc.tensor.transpose(
                            ptr[:, qq, :],
                            ot[0 : D + 2, it * 128 : (it + 1) * 128],
                            id_sb[0 : D + 2, 0 : D + 2],
                        )
                    g_sl = slice(grp * 4, grp * 4 + 4)
                    nc.vector.reciprocal_approx_fast(rcol[:, g_sl], ptr[:, :, D + 1])
                    rc4 = rcol[:, g_sl]
                    rcol_bc = bass.AP(
                        tensor=rcol.tensor, offset=rc4.offset,
                        ap=[rcol.ap[0], [1, 4], [0, D]],
                    )
                    nc.vector.tensor_tensor(
                        out=y[:, g_sl, :], in0=ptr[:, :, 0:D], in1=rcol_bc,
                        op=ALU.mult,
                    )
                    nc.vector.tensor_tensor(
                        out=mus[:, g_sl], in0=ptr[:, :, D], in1=rc4, op=ALU.mult
                    )
                nc.vector.tensor_add(y, y, xr)

                # ---- LayerNorm stats: mean from the free sum_d num column,
                # variance from one squared reduce ----
                ysq = yp.tile([128, NT, D], DT.bfloat16, tag="ysq")
                nc.vector.tensor_mul(ysq, y, y)
                ys2 = statp.tile([128, NT], DT.float32, tag="ys2")
                nc.vector.reduce_sum(ys2, ysq, axis=mybir.AxisListType.X)
                mu = statp.tile([128, NT], DT.float32, tag="mu")
                nc.gpsimd.tensor_scalar(
                    mu, mus, scalar1=1.0 / D, scalar2=None, op0=ALU.mult
                )
                nc.gpsimd.tensor_add(mu, mu, xs)
                # var + eps = ys2/64 - mu^2 + 1e-5
                m2 = statp.tile([128, NT], DT.float32, tag="m2")
                nc.gpsimd.tensor_mul(m2, mu, mu)
                ve = statp.tile([128, NT], DT.float32, tag="ve")
                nc.gpsimd.tensor_scalar(
                    ve, ys2, scalar1=1.0 / D, scalar2=1e-5, op0=ALU.mult, op1=ALU.add
                )
                nc.gpsimd.tensor_sub(ve, ve, m2)
                sd = statp.tile([128, NT], DT.float32, tag="sd")
                nc.scalar.activation(sd, ve, AF.Sqrt)
                rstd = statp.tile([128, NT], DT.float32, tag="rstd")
                nc.vector.reciprocal_approx_fast(rstd, sd)
                mub = statp.tile([128, NT], DT.bfloat16, tag="mub")
                nc.vector.tensor_copy(mub, mu)
                rstdb = statp.tile([128, NT], DT.bfloat16, tag="rstdb")
                nc.vector.tensor_copy(rstdb, rstd)

                # ---- normalize per i-tile on Pool: out = (y - mu) * rstd ----
                out_sb = outp.tile([128, NT, D], DT.bfloat16, tag="o")
                mub_bc = bass.AP(
                    tensor=mub.tensor, offset=mub.offset,
                    ap=[mub.ap[0], [1, NT], [0, D]],
                )
                rstdb_bc = bass.AP(
                    tensor=rstdb.tensor, offset=rstdb.offset,
                    ap=[rstdb.ap[0], [1, NT], [0, D]],
                )
                nc.vector.tensor_tensor(
                    out=out_sb, in0=y, in1=mub_bc, op=ALU.subtract
                )
                nc.vector.tensor_tensor(
                    out=out_sb, in0=out_sb, in1=rstdb_bc, op=ALU.mult
                )
                if apply_gamma_beta:
                    for t in range(NT):
                        nc.gpsimd.tensor_mul(out_sb[:, t, :], out_sb[:, t, :], gam_sb)
                        nc.gpsimd.tensor_add(out_sb[:, t, :], out_sb[:, t, :], bet_sb)

                nc.sync.dma_start(
                    out=out_t[b].rearrange("(t p) d -> p t d", p=128), in_=out_sb
                )

    nc.compile()
    return nc


_NC_CACHE: dict = {}


def kernel(input1, V_w, V_b, phi, ln_gamma, ln_beta, _trace=False):
    input1 = np.ascontiguousarray(np.asarray(input1, dtype=np.float32))
    V_w = np.asarray(V_w, dtype=np.float32)
    V_b = np.asarray(V_b, dtype=np.float32)
    phi = np.asarray(phi, dtype=np.float32)
    ln_gamma = np.asarray(ln_gamma, dtype=np.float32)
    ln_beta = np.asarray(ln_beta, dtype=np.float32)

    apply_gb = not (np.all(ln_gamma == 1.0) and np.all(ln_beta == 0.0))

    if apply_gb not in _NC_CACHE:
        _NC_CACHE[apply_gb] = _build_nc(apply_gb)
    nc = _NC_CACHE[apply_gb]

    O = _build_orthogonal(phi)
    vwT = V_w.T.astype(np.float32)
    vw16 = np.ascontiguousarray(
        np.concatenate([vwT, vwT.sum(axis=1, keepdims=True)], axis=1)
        .astype(np.float16)
    )
    ident = np.eye(128, dtype=np.float32)
    gb = np.ascontiguousarray(np.stack([ln_gamma, ln_beta]).astype(np.float32))

    # host prep: W' = x * ||x||^-1/2 * 8^-1/4 * (128/ln2)^1/4 (transposed f16),
    # gt = (W' O)^T, xo = X^T, xr = x + V_b (bf16 residual w/ folded bias)
    ss = (input1.astype(np.float64) ** 2).sum(-1)
    s = (ss ** -0.25 * 8.0 ** -0.25 * SCHRAUD_C4 ** 0.25).astype(np.float32)
    w = input1 * s[..., None]
    wt_full = np.ascontiguousarray(w.transpose(0, 2, 1).astype(np.float16))
    gt_full = np.ascontiguousarray(
        (w @ O).transpose(0, 2, 1).astype(np.float16)
    )
    xo_full = np.ascontiguousarray(
        input1.transpose(0, 2, 1).astype(np.float16)
    )
    xr_f32 = input1 + V_b
    xr_full = np.ascontiguousarray(xr_f32.astype(ml_dtypes.bfloat16))
    xs_full = np.ascontiguousarray((xr_f32.sum(-1) / D).astype(np.float32))

    in_maps = []
    for c in range(NCORES):
        sl = slice(c * BPC, (c + 1) * BPC)
        in_maps.append(
            {
                "wt": wt_full[sl],
                "gt": gt_full[sl],
                "xo": xo_full[sl],
                "xr": xr_full[sl],
                "xs": xs_full[sl],
                "vw": vw16,
                "ident": ident,
                "gb": gb,
            }
        )

    res = bass_utils.run_bass_kernel_spmd(
        nc, in_maps, core_ids=list(range(NCORES)), trace=_trace
    )
    out = np.concatenate(
        [res.results[c]["out"].astype(np.float32) for c in range(NCORES)], axis=0
    )
    if _trace:
        kernel._last_result = res
    return out
